# revision 39
# baseline (speedup 1.0000x reference)
"""Trainium2 Bass kernel for nn_FLASH_40458591928592 (sparse_attention).

Sequence-sharded over 8 NeuronCores: 1024 tokens (= 4 groups of 256) per
core. Big GEMMs (qk/gate/y) bf16; the v/kv/quad chain runs in fp8(e4m3)
DoubleRow (2x PE throughput, contraction 256 per instruction):

  v    = silu(xh8 @ wv8)/32    1-term fp8 DR (wv pre-scaled x32 on host,
                               e4m3's subnormal floor), stored e4m3
  kv   = lk8 @ v8              fp8 DR over the two token-tiles of a group
  quad = attn8(x1024) @ v8     fp8 DR; causal-conv band folded into attn8
                               on DVE; cross-group tail via 32-col matmul
  gate/qk/y = bf16 as before

Single full-width lin_kv pass -> one AllGather (bf16) fired ~45us in,
hidden behind qk/halo/quad + gate(stream0-th0) silu work; weighted
prefix sum of the gathered totals split across DVE and GpSimd.
"""

from contextlib import ExitStack

import numpy as np
import ml_dtypes

import concourse.tile as tile
from concourse import bacc, mybir
from concourse.bass_utils import run_bass_kernel_spmd
from concourse.masks import make_identity

BF = mybir.dt.bfloat16
F32 = mybir.dt.float32
E4 = mybir.dt.float8e4
bf16 = ml_dtypes.bfloat16
e4m3 = ml_dtypes.float8_e4m3fn

G = 256
DIM = 1024
HID = 2048
DQK = 128
NSEQ = 8192
NC = 8
T = NSEQ // NC        # 1024 tokens per core
NG = T // G           # 4 groups per core
KD = DIM // 128       # 8 k-tiles over dim
KP = KD // 2          # 4 k-pairs (DoubleRow)
ET = HID // 128       # 16 e-tiles over hid
TT = T // 128         # 8 token tiles

WS = 32.0             # wv pre-scale (host) -> 1/WS at PSUM readout
ASC = 1024.0          # attn scale for fp8
RASC = float(np.sqrt(ASC))

AF = mybir.ActivationFunctionType
ALU = mybir.AluOpType
DR = mybir.MatmulPerfMode.DoubleRow

DEBUG_DUMPS = False
WITH_VBIAS = True     # ones-row matmuls for b_h[:HID] (skipped when zero)
WITH_OBIAS = True     # ones-row matmuls for b_out (skipped when zero)


def _emit(tc, ap):
    nc = tc.nc
    with ExitStack() as ctx:
        consts = ctx.enter_context(tc.tile_pool(name="consts", bufs=1))
        p_x8 = ctx.enter_context(tc.tile_pool(name="x8", bufs=1))
        p_xt = ctx.enter_context(tc.tile_pool(name="xt", bufs=4))
        p_qkT = ctx.enter_context(tc.tile_pool(name="qkT", bufs=1))
        p_lk = ctx.enter_context(tc.tile_pool(name="lk", bufs=1))
        p_w = ctx.enter_context(tc.tile_pool(name="w", bufs=1))    # wv8 -> wg
        p_S = ctx.enter_context(tc.tile_pool(name="S", bufs=1))    # S_full -> wout
        p_v = ctx.enter_context(tc.tile_pool(name="v", bufs=2))    # v8 -> og ring
        p_tails = ctx.enter_context(tc.tile_pool(name="tails", bufs=1))
        p_outT = ctx.enter_context(tc.tile_pool(name="outT", bufs=1))
        p_a0 = ctx.enter_context(tc.tile_pool(name="a0", bufs=2))
        p_a1 = ctx.enter_context(tc.tile_pool(name="a1", bufs=1))
        p_sq = ctx.enter_context(tc.tile_pool(name="sq", bufs=1))
        p_at = ctx.enter_context(tc.tile_pool(name="at", bufs=4))
        p_tr = ctx.enter_context(tc.tile_pool(name="tr", bufs=2))
        p_y = ctx.enter_context(tc.tile_pool(name="ysb", bufs=2))
        p_xr = ctx.enter_context(tc.tile_pool(name="xr", bufs=2))
        p_st = ctx.enter_context(tc.tile_pool(name="st", bufs=2))
        ps1 = ctx.enter_context(tc.tile_pool(name="ps1", bufs=4, space="PSUM"))
        ps2 = ctx.enter_context(tc.tile_pool(name="ps2", bufs=2, space="PSUM"))
        pskv = ctx.enter_context(tc.tile_pool(name="pskv", bufs=2, space="PSUM"))

        # ---- weight/const DMAs on the ACT HWDGE queue ----
        wv = p_w.tile([128, KD, HID], E4, tag="w")
        nc.scalar.dma_start(wv, ap["wv8"].rearrange("(kt p) e -> p kt e", p=128))
        wqk_sb = consts.tile([128, KD, DQK], BF, tag="wqk")
        nc.scalar.dma_start(wqk_sb, ap["wqk"].rearrange("(kt p) q -> p kt q", p=128))
        xhalo = consts.tile([128, KD, 128], E4, tag="xhalo")
        nc.scalar.dma_start(xhalo, ap["xhalo8"].rearrange("(kt p) t -> p kt t", p=128))
        bqk = consts.tile([128, 1], F32, tag="bqk")
        nc.scalar.dma_start(bqk, ap["bqk"])

        # fp8 x for stream 0 (v GEMM); host pre-arranged [128, KD, T] so
        # each DMA line is a full 8KB contiguous partition row
        x8 = p_x8.tile([128, KD, T], E4, tag="x8")
        for q in range(2):
            nc.sync.dma_start(x8[:, q * 4:(q + 1) * 4, :],
                              ap["xh8"][:, q * 4:(q + 1) * 4, :])

        def load_xt(s):
            halves = []
            for h in range(2):
                t = p_xt.tile([128, KD // 2, T], BF, tag="xt")
                for q in range(2):
                    r0 = h * 512 + q * 256
                    nc.sync.dma_start(
                        t[:, q * 2:(q + 1) * 2, :],
                        ap["xt"][s, r0:r0 + 256, :].rearrange(
                            "(kt p) t -> p kt t", p=128))
                halves.append(t)
            return halves

        def xt_k(halves, kt):
            return halves[kt // 4][:, kt % 4, :]

        x_s0 = load_xt(0)

        # warm-up collective: absorbs first-collective setup latency
        warm = consts.tile([128, 16], BF, tag="warm")
        nc.vector.memset(warm, 0.0)
        nc.sync.dma_start(ap["cc_warm_in"], warm)
        nc.gpsimd.collective_compute(
            "AllGather", ALU.bypass, replica_groups=[list(range(NC))],
            ins=[ap["cc_warm_in"]], outs=[ap["cc_warm_out"]])

        # ---- v = silu(x8 @ wv)/WS  (1-term fp8 DR), stored e4m3 ----
        if WITH_VBIAS or WITH_OBIAS:
            ones_t = consts.tile([1, 128], BF, tag="ones")
            nc.vector.memset(ones_t, 1.0)
        if WITH_VBIAS:
            wvb = consts.tile([1, HID], BF, tag="wvb")
            nc.scalar.dma_start(wvb, ap["wvb"])

        x_s3 = load_xt(3)
        v8 = p_v.tile([128, TT, HID], E4, tag="v")
        for tt in range(TT):
            for c in range(4):
                c0 = c * 512
                ps = ps1.tile([128, 512], F32, tag="ps1")
                for kp in range(KP):
                    nc.tensor.matmul(
                        ps, x8[:, 2 * kp:2 * kp + 2, tt * 128:(tt + 1) * 128],
                        wv[:, 2 * kp:2 * kp + 2, c0:c0 + 512],
                        start=(kp == 0), stop=(kp == KP - 1 and not WITH_VBIAS),
                        perf_mode=DR, skip_group_check=True)
                if WITH_VBIAS:
                    nc.tensor.matmul(ps, ones_t[0:1, 0:128],
                                     wvb[0:1, c0:c0 + 512],
                                     start=False, stop=True,
                                     skip_group_check=True)
                # ACT writes bf16; DVE does the e4m3 cast (ACT fp8 output
                # untested on this hw)
                stg = p_st.tile([128, 512], BF, tag="st")
                nc.scalar.activation(stg, ps, AF.Silu, bias=0.0, scale=1.0 / WS)
                nc.vector.tensor_copy(v8[:, tt, c0:c0 + 512], stg)

        # ---- qk streams (bf16); s0 and s3 first (kv needs lk from s3) ----
        qkT = p_qkT.tile([128, 4, T], BF, tag="qkT")

        def qk_stream(s, halves):
            for ch in range(2):
                ps = ps1.tile([128, 512], F32, tag="ps1")
                for kt in range(KD):
                    nc.tensor.matmul(
                        ps, wqk_sb[:, kt, :],
                        xt_k(halves, kt)[:, ch * 512:(ch + 1) * 512],
                        start=(kt == 0), stop=(kt == KD - 1))
                nc.scalar.activation(qkT[:, s, ch * 512:(ch + 1) * 512], ps,
                                     AF.Silu, bias=bqk, scale=1.0)

        qk_stream(0, x_s0)
        qk_stream(3, x_s3)

        # lk (stream 3) token-major via PE transpose, stored e4m3
        ident = consts.tile([128, 128], BF, tag="ident")
        make_identity(nc, ident)
        lk8 = p_lk.tile([128, TT, 128], E4, tag="lk")
        for tt in range(TT):
            pt = ps2.tile([128, 128], BF, tag="ps2")
            nc.tensor.transpose(pt, qkT[:, 3, tt * 128:(tt + 1) * 128], ident)
            nc.vector.tensor_copy(lk8[:, tt, :], pt)

        # ---- kv sweep (fp8 DR over token-tile pairs) + snapshots ----
        # local full-sum totals stage in S_full[:, 0, :] (dead until the
        # post-AllGather weighted sum overwrites it)
        S_full = p_S.tile([128, NG, HID], BF, tag="S")

        for half in range(2):
            pk0 = pskv.tile([128, 512], F32, tag="pskv")
            pk1 = pskv.tile([128, 512], F32, tag="pskv")
            pk = [pk0, pk1]
            for g in range(NG):
                for c in range(2):
                    c0 = half * 1024 + c * 512
                    nc.tensor.matmul(pk[c], lk8[:, 2 * g:2 * g + 2, :],
                                     v8[:, 2 * g:2 * g + 2, c0:c0 + 512],
                                     start=(g == 0), stop=(g == NG - 1),
                                     perf_mode=DR, skip_group_check=True)
                    dst = S_full[:, (g + 1) % NG, c0:c0 + 512]
                    nc.scalar.activation(dst, pk[c], AF.Copy, bias=0.0,
                                         scale=1.0 / G)

        # AllGather the per-core totals
        nc.sync.dma_start(ap["cc_in"], S_full[:, 0, :])
        nc.gpsimd.collective_compute(
            "AllGather", ALU.bypass, replica_groups=[list(range(NC))],
            ins=[ap["cc_in"]], outs=[ap["cc_out"]])

        # ---- halo tail (last 32 tokens of previous core, padded to 128
        # tokens so the DR shapes match the v chains), masked on core 0 ----
        tails = p_tails.tile([32, NG, HID], E4, tag="tails")
        hmask = consts.tile([32, 1], F32, tag="hmask")
        nc.scalar.dma_start(hmask, ap["hmask"])
        for c in range(4):
            c0 = c * 512
            ps = ps1.tile([128, 512], F32, tag="ps1")
            for kp in range(KP):
                nc.tensor.matmul(ps, xhalo[:, 2 * kp:2 * kp + 2, :],
                                 wv[:, 2 * kp:2 * kp + 2, c0:c0 + 512],
                                 start=(kp == 0),
                                 stop=(kp == KP - 1 and not WITH_VBIAS),
                                 perf_mode=DR, skip_group_check=True)
            if WITH_VBIAS:
                nc.tensor.matmul(ps, ones_t[0:1, 0:128], wvb[0:1, c0:c0 + 512],
                                 start=False, stop=True, skip_group_check=True)
            stg = p_st.tile([128, 512], BF, tag="st")
            nc.scalar.activation(stg, ps, AF.Silu, bias=0.0, scale=1.0 / WS)
            nc.vector.tensor_scalar_mul(tails[:, 0, c0:c0 + 512],
                                        stg[0:32, :], hmask)
        # group tails: last 32 tokens of the previous group (partition remap)
        for g in range(1, NG):
            nc.sync.dma_start(tails[:, g, :], v8[96:128, 2 * g - 1, :])

        # ---- remaining qk streams ----
        x_s2 = load_xt(2)
        qk_stream(2, x_s2)
        x_s1 = load_xt(1)
        qk_stream(1, x_s1)

        # ---- sim / attn8 (conv band folded in, scaled by ASC, e4m3) ----
        triu = consts.tile([128, 128], BF, tag="triu")
        nc.scalar.dma_start(triu, ap["triu"])
        band0 = consts.tile([128, 256], BF, tag="band0")
        nc.scalar.dma_start(band0, ap["band0"])
        band1 = consts.tile([128, 128], BF, tag="band1")
        nc.scalar.dma_start(band1, ap["band1"])
        bprev = consts.tile([32, 32], E4, tag="bprev")
        nc.scalar.dma_start(bprev, ap["bprev"])

        attn8s = []
        for g in range(NG):
            i0 = g * G
            a0 = p_a0.tile([128, 256], BF, tag="a0")
            ps = ps2.tile([128, 256], F32, tag="ps2")
            nc.tensor.matmul(ps, qkT[:, 2, i0:i0 + 128], qkT[:, 0, i0:i0 + 256],
                             start=True, stop=True)
            nc.scalar.activation(a0, ps, AF.Relu, bias=0.0, scale=RASC / G)
            nc.vector.tensor_mul(a0[:, 0:128], a0[:, 0:128], triu)

            a1 = p_a1.tile([128, 128], BF, tag="a1")
            ps = ps2.tile([128, 256], F32, tag="ps2")
            nc.tensor.matmul(ps[:, 0:128], qkT[:, 2, i0 + 128:i0 + 256],
                             qkT[:, 0, i0 + 128:i0 + 256], start=True, stop=True)
            nc.scalar.activation(a1, ps[:, 0:128], AF.Relu, bias=0.0,
                                 scale=RASC / G)
            nc.vector.tensor_mul(a1, a1, triu)

            t8 = p_at.tile([128, 2, 256], E4, tag="at")
            sq = p_sq.tile([128, 256], BF, tag="sq")
            nc.vector.tensor_mul(sq, a0, a0)
            nc.vector.scalar_tensor_tensor(t8[:, 0, :], sq, 1.0, band0,
                                           op0=ALU.mult, op1=ALU.add)
            nc.vector.tensor_scalar_mul(t8[:, 1, 0:128], triu, 0.0)
            sq1 = p_sq.tile([128, 256], BF, tag="sq")
            nc.vector.tensor_mul(sq1[:, 0:128], a1, a1)
            nc.vector.scalar_tensor_tensor(t8[:, 1, 128:256], sq1[:, 0:128],
                                           1.0, band1, op0=ALU.mult, op1=ALU.add)
            attn8s.append(t8)

        # ---- quad + conv -> outT (fp8 DR + 32-col tail matmul) ----
        outT = p_outT.tile([128, ET, T], BF, tag="outT")
        for gp in range(NG // 2):
            for et in range(ET):
                e0 = et * 128
                po = ps2.tile([128, 512], F32, tag="ps2")
                for h in range(2):
                    g = 2 * gp + h
                    c0 = h * 256
                    nc.tensor.matmul(po[:, c0:c0 + 256],
                                     v8[:, 2 * g:2 * g + 2, e0:e0 + 128],
                                     attn8s[g], start=True, stop=False,
                                     perf_mode=DR, skip_group_check=True)
                    nc.tensor.matmul(po[:, c0:c0 + 32],
                                     tails[:, g, e0:e0 + 128],
                                     bprev[0:32, :], start=False, stop=True,
                                     skip_group_check=True)
                nc.scalar.activation(outT[:, et, 2 * gp * G:(2 * gp + 2) * G],
                                     po, AF.Copy, bias=0.0, scale=1.0 / ASC)

        # ---- weighted prefix sum of AllGathered totals (DVE), emitted
        # early so it runs concurrent with quad/gate PE work ----
        wsumw = consts.tile([128, NC], F32, tag="wsumw")
        nc.scalar.dma_start(wsumw, ap["wsumw"])
        for r in range(NC):
            for h in range(2):
                h0 = h * 1024
                trt = p_tr.tile([128, 1024], BF, tag="tr")
                nc.sync.dma_start(
                    trt, ap["cc_out"][r * 128:(r + 1) * 128, h0:h0 + 1024])
                if r == 0:
                    nc.vector.tensor_scalar_mul(S_full[:, 0, h0:h0 + 1024],
                                                trt, wsumw[:, 0:1])
                else:
                    nc.vector.scalar_tensor_tensor(
                        S_full[:, 0, h0:h0 + 1024], trt, wsumw[:, r:r + 1],
                        S_full[:, 0, h0:h0 + 1024], op0=ALU.mult, op1=ALU.add)
        for g in range(1, NG):
            nc.vector.tensor_add(S_full[:, g, :], S_full[:, g, :],
                                 S_full[:, 0, :])

        # ---- gate weights + consts (DMA while PE chews on quad) ----
        wg_sb = p_w.tile([128, KD, HID], BF, tag="w")
        for kt in range(KD):
            nc.scalar.dma_start(wg_sb[:, kt, :],
                                ap["wg"][kt * 128:(kt + 1) * 128, :])
        bgate = consts.tile([128, ET], F32, tag="bgate")
        nc.scalar.dma_start(bgate, ap["bgate"])

        def gate_th(s, halves, th, og, defer_muls=False):
            t0 = th * 512
            for et in range(ET):
                ps = ps1.tile([128, 512], F32, tag="ps1")
                for kt in range(KD):
                    nc.tensor.matmul(
                        ps, wg_sb[:, kt, et * 128:(et + 1) * 128],
                        xt_k(halves, kt)[:, t0:t0 + 512],
                        start=(kt == 0), stop=(kt == KD - 1))
                nc.scalar.activation(og[:, et, :], ps, AF.Silu,
                                     bias=bgate[:, et:et + 1], scale=1.0)
                if not defer_muls:
                    nc.vector.tensor_mul(og[:, et, :], og[:, et, :],
                                         outT[:, et, t0:t0 + 512])

        xg0 = load_xt(0)
        og0 = p_v.tile([128, ET, 512], BF, tag="v")
        gate_th(0, xg0, 0, og0, defer_muls=True)

        # scheduler fence: keep the lin matmuls (which wait on the
        # AllGather) from being hoisted ahead of the pre-fence PE work
        tc.no_sync_barrier()

        # ---- lin: outT += S_full[g] @ lq, et-major with og0 muls chasing ----
        for et in range(ET):
            e0 = et * 128
            for g in range(NG):
                po = ps2.tile([128, 256], F32, tag="ps2")
                nc.tensor.matmul(po, S_full[:, g, e0:e0 + 128],
                                 qkT[:, 1, g * G:(g + 1) * G],
                                 start=True, stop=True)
                nc.vector.tensor_add(outT[:, et, g * G:(g + 1) * G],
                                     outT[:, et, g * G:(g + 1) * G], po)
            nc.vector.tensor_mul(og0[:, et, :], og0[:, et, :],
                                 outT[:, et, 0:512])

        if DEBUG_DUMPS:
            nc.sync.dma_start(ap["dbg_v8"], v8)
            nc.sync.dma_start(ap["dbg_qkT"], qkT)
            nc.sync.dma_start(ap["dbg_outT"], outT)
            nc.sync.dma_start(ap["dbg_sfull"], S_full)
            nc.sync.dma_start(ap["dbg_tails"], tails)
            for g in range(NG):
                nc.sync.dma_start(ap["dbg_attn8"][g], attn8s[g])

        # ---- y projection (bf16) ----
        wout_sb = p_S.tile([128, ET, DIM], BF, tag="S")
        for kt in range(ET):
            nc.scalar.dma_start(wout_sb[:, kt, :],
                                ap["wout"][kt * 128:(kt + 1) * 128, :])
        if WITH_OBIAS:
            bout = consts.tile([1, DIM], BF, tag="bout")
            nc.scalar.dma_start(bout, ap["bout"])

        def y_th(s, og, th):
            for tl in range(4):
                tt = th * 4 + tl
                for nch in range(2):
                    n0 = nch * 512
                    ps = ps1.tile([128, 512], F32, tag="ps1")
                    for kt in range(ET):
                        nc.tensor.matmul(
                            ps, og[:, kt, tl * 128:(tl + 1) * 128],
                            wout_sb[:, kt, n0:n0 + 512],
                            start=(kt == 0),
                            stop=(kt == ET - 1 and not WITH_OBIAS),
                            skip_group_check=True)
                    if WITH_OBIAS:
                        nc.tensor.matmul(ps, ones_t[0:1, 0:128],
                                         bout[0:1, n0:n0 + 512],
                                         start=False, stop=True,
                                         skip_group_check=True)
                    xr = p_xr.tile([128, 512], F32, tag="xr")
                    nc.sync.dma_start(
                        xr, ap["xtok"][s, tt * 128:(tt + 1) * 128, n0:n0 + 512])
                    ysb = p_y.tile([128, 512], F32, tag="ysb")
                    nc.vector.scalar_tensor_tensor(
                        ysb, ps, 0.0, xr, op0=ALU.add, op1=ALU.add)
                    nc.sync.dma_start(
                        ap["y"][s, tt * 128:(tt + 1) * 128, n0:n0 + 512], ysb)

        # gate s0-th1 runs on PE while DVE finishes the og0 muls
        og1 = p_v.tile([128, ET, 512], BF, tag="v")
        gate_th(0, xg0, 1, og1)
        y_th(0, og0, 0)
        y_th(0, og1, 1)
        xg = {}
        for s in range(1, 4):
            halves = xg[s] if s in xg else load_xt(s)
            if s + 1 < 4:
                xg[s + 1] = load_xt(s + 1)
            for th in range(2):
                og = p_v.tile([128, ET, 512], BF, tag="v")
                gate_th(s, halves, th, og)
                y_th(s, og, th)


def build_nc(with_vbias=None, with_obias=None):
    global WITH_VBIAS, WITH_OBIAS
    if with_vbias is not None:
        WITH_VBIAS = with_vbias
    if with_obias is not None:
        WITH_OBIAS = with_obias
    nc = bacc.Bacc("TRN2", target_bir_lowering=False, debug=False, num_devices=NC)
    ap = {}

    def dram(name, shape, dt, kind=None, addr_space=None):
        kw = {}
        if kind:
            kw["kind"] = kind
        if addr_space:
            kw["addr_space"] = addr_space
        ap[name] = nc.dram_tensor(name, shape, dt, **kw).ap()

    dram("xt", [4, DIM, T], BF, kind="ExternalInput")
    dram("xh8", [128, KD, T], E4, kind="ExternalInput")
    dram("xhalo8", [DIM, 128], E4, kind="ExternalInput")
    dram("xtok", [4, T, DIM], F32, kind="ExternalInput")
    dram("wv8", [DIM, HID], E4, kind="ExternalInput")
    dram("wg", [DIM, HID], BF, kind="ExternalInput")
    dram("wqk", [DIM, DQK], BF, kind="ExternalInput")
    dram("wout", [HID, DIM], BF, kind="ExternalInput")
    dram("wvb", [1, HID], BF, kind="ExternalInput")
    dram("bout", [1, DIM], BF, kind="ExternalInput")
    dram("bgate", [128, ET], F32, kind="ExternalInput")
    dram("bqk", [128, 1], F32, kind="ExternalInput")
    dram("triu", [128, 128], BF, kind="ExternalInput")
    dram("band0", [128, 256], BF, kind="ExternalInput")
    dram("band1", [128, 128], BF, kind="ExternalInput")
    dram("bprev", [32, 32], E4, kind="ExternalInput")
    dram("hmask", [32, 1], F32, kind="ExternalInput")
    dram("wsumw", [128, NC], F32, kind="ExternalInput")
    if DEBUG_DUMPS:
        dram("dbg_v8", [128, TT, HID], E4, kind="ExternalOutput")
        dram("dbg_qkT", [128, 4, T], BF, kind="ExternalOutput")
        dram("dbg_outT", [128, ET, T], BF, kind="ExternalOutput")
        dram("dbg_sfull", [128, NG, HID], BF, kind="ExternalOutput")
        dram("dbg_tails", [32, NG, HID], E4, kind="ExternalOutput")
        dram("dbg_attn8", [NG, 128, 2, 256], E4, kind="ExternalOutput")
    dram("cc_warm_in", [128, 16], BF)
    dram("cc_warm_out", [NC * 128, 16], BF, addr_space="Shared")
    dram("cc_in", [128, HID], BF)
    dram("cc_out", [NC * 128, HID], BF, addr_space="Shared")
    dram("y", [4, T, DIM], F32, kind="ExternalOutput")

    with tile.TileContext(nc) as tc:
        _emit(tc, ap)
    nc.compile()
    return nc


def host_prep(inputs):
    """Pure layout transforms: shard, transpose, fp8 casts, conv-band consts."""
    x = np.ascontiguousarray(np.asarray(inputs["x"], np.float32)[0])  # [4,N,DIM]
    W_h = np.asarray(inputs["W_h"], np.float32)
    b_h = np.asarray(inputs["b_h"], np.float32)
    W_qk = np.asarray(inputs["W_qk"], np.float32)
    b_qk = np.asarray(inputs["b_qk"], np.float32)
    W_out = np.asarray(inputs["W_out"], np.float32)
    b_out = np.asarray(inputs["b_out"], np.float32)
    cw = np.asarray(inputs["conv_w"], np.float32)

    jj = np.arange(128)[:, None]
    ii = np.arange(128)[None, :]
    d = ii - jj
    triu = (ii >= jj).astype(bf16)
    w63 = cw * (np.arange(63) <= 31)
    bdiag = np.where((d >= 0) & (d <= 31), w63[np.clip(31 - d, 0, 62)], 0.0)
    dc = (ii + 128) - jj
    bcorn = np.where((dc >= 0) & (dc <= 31), w63[np.clip(31 - dc, 0, 62)], 0.0)
    band0 = (np.concatenate([bdiag, bcorn], axis=1) * ASC).astype(bf16)
    band1 = (bdiag * ASC).astype(bf16)
    jt = np.arange(32)[:, None]
    ip = np.arange(32)[None, :]
    dp = ip + 32 - jt
    bprev = np.where((dp >= 1) & (dp <= 31),
                     w63[np.clip(31 - dp, 0, 62)], 0.0) * ASC

    common = {
        "wv8": (np.ascontiguousarray(W_h[:, :HID]) * WS).astype(e4m3),
        "wg": np.ascontiguousarray(W_h[:, HID:]).astype(bf16),
        "wqk": W_qk.astype(bf16),
        "wout": W_out.astype(bf16),
        "wvb": (b_h[None, :HID] * WS).astype(bf16),
        "bout": b_out[None, :].astype(bf16),
        "bgate": np.ascontiguousarray(
            b_h[HID:].reshape(ET, 128).T).astype(np.float32),
        "bqk": b_qk[:, None].astype(np.float32),
        "triu": triu, "band0": band0, "band1": band1,
        "bprev": bprev.astype(e4m3),
    }

    in_maps = []
    for c in range(NC):
        sl = slice(c * T, (c + 1) * T)
        x_c = x[:, sl, :]
        xt = np.ascontiguousarray(x_c.transpose(0, 2, 1))   # [4, DIM, T]
        xh = np.zeros((DIM, 128), np.float32)
        if c > 0:
            xh[:, 0:32] = x[0, c * T - 32:c * T, :].T
        m = dict(common)
        m["xt"] = xt.astype(bf16)
        m["xh8"] = np.ascontiguousarray(
            xt[0].astype(e4m3).reshape(KD, 128, T).swapaxes(0, 1))
        m["xhalo8"] = xh.astype(e4m3)
        m["xtok"] = np.ascontiguousarray(x_c)
        m["hmask"] = np.full((32, 1), 1.0 if c > 0 else 0.0, np.float32)
        w = np.zeros((128, NC), np.float32)
        w[:, :c] = 1.0
        m["wsumw"] = w
        in_maps.append(m)
    return in_maps


_NC_PROG = None
_NC_FLAGS = None


def kernel(**inputs):
    global _NC_PROG, _NC_FLAGS
    b_h = np.asarray(inputs["b_h"], np.float32)
    b_out = np.asarray(inputs["b_out"], np.float32)
    flags = (bool(np.any(b_h[:HID])), bool(np.any(b_out)))
    if _NC_PROG is None or _NC_FLAGS != flags:
        _NC_PROG = build_nc(with_vbias=flags[0], with_obias=flags[1])
        _NC_FLAGS = flags
    in_maps = host_prep(inputs)
    res = run_bass_kernel_spmd(_NC_PROG, in_maps, list(range(NC)))
    y = np.stack([res.results[c]["y"] for c in range(NC)], axis=1)
    return np.ascontiguousarray(y.reshape(4, NSEQ, DIM)[None]).astype(np.float32)


# revision 40
# speedup vs baseline: 1.0538x; 1.0538x over previous
"""Trainium2 Bass kernel for nn_FLASH_40458591928592 (sparse_attention).

Sequence-sharded over 8 NeuronCores: 1024 tokens (= 4 groups of 256) per
core. Big GEMMs (qk/gate/y) bf16; the v/kv/quad chain runs in fp8(e4m3)
DoubleRow (2x PE throughput, contraction 256 per instruction):

  v    = silu(xh8 @ wv8)/32    1-term fp8 DR (wv pre-scaled x32 on host,
                               e4m3's subnormal floor), stored e4m3
  kv   = lk8 @ v8              fp8 DR over the two token-tiles of a group
  quad = attn8(x1024) @ v8     fp8 DR; causal-conv band folded into attn8
                               on DVE; cross-group tail via 32-col matmul
  gate/qk/y = bf16 as before

Single full-width lin_kv pass -> one AllGather (bf16) fired ~45us in,
hidden behind qk/halo/quad + gate(stream0-th0) silu work; weighted
prefix sum of the gathered totals split across DVE and GpSimd.
"""

from contextlib import ExitStack

import numpy as np
import ml_dtypes

import concourse.tile as tile
from concourse import bacc, mybir
from concourse.bass_utils import run_bass_kernel_spmd
from concourse.masks import make_identity

BF = mybir.dt.bfloat16
F32 = mybir.dt.float32
E4 = mybir.dt.float8e4
bf16 = ml_dtypes.bfloat16
e4m3 = ml_dtypes.float8_e4m3fn

G = 256
DIM = 1024
HID = 2048
DQK = 128
NSEQ = 8192
NC = 8
T = NSEQ // NC        # 1024 tokens per core
NG = T // G           # 4 groups per core
KD = DIM // 128       # 8 k-tiles over dim
KP = KD // 2          # 4 k-pairs (DoubleRow)
ET = HID // 128       # 16 e-tiles over hid
TT = T // 128         # 8 token tiles

WS = 32.0             # wv pre-scale (host) -> 1/WS at PSUM readout
ASC = 1024.0          # attn scale for fp8
RASC = float(np.sqrt(ASC))

AF = mybir.ActivationFunctionType
ALU = mybir.AluOpType
DR = mybir.MatmulPerfMode.DoubleRow

DEBUG_DUMPS = False
WITH_VBIAS = True     # ones-row matmuls for b_h[:HID] (skipped when zero)
WITH_OBIAS = True     # ones-row matmuls for b_out (skipped when zero)


def _emit(tc, ap):
    nc = tc.nc
    with ExitStack() as ctx:
        consts = ctx.enter_context(tc.tile_pool(name="consts", bufs=1))
        p_x8 = ctx.enter_context(tc.tile_pool(name="x8", bufs=1))
        p_xt = ctx.enter_context(tc.tile_pool(name="xt", bufs=4))
        p_qkT = ctx.enter_context(tc.tile_pool(name="qkT", bufs=1))
        p_lk = ctx.enter_context(tc.tile_pool(name="lk", bufs=1))
        p_w = ctx.enter_context(tc.tile_pool(name="w", bufs=1))    # wv8 -> wg
        p_S = ctx.enter_context(tc.tile_pool(name="S", bufs=1))    # S_full -> wout
        p_v = ctx.enter_context(tc.tile_pool(name="v", bufs=2))    # v8 -> og ring
        p_tails = ctx.enter_context(tc.tile_pool(name="tails", bufs=1))
        p_outT = ctx.enter_context(tc.tile_pool(name="outT", bufs=1))
        p_a0 = ctx.enter_context(tc.tile_pool(name="a0", bufs=2))
        p_a1 = ctx.enter_context(tc.tile_pool(name="a1", bufs=1))
        p_sq = ctx.enter_context(tc.tile_pool(name="sq", bufs=1))
        p_at = ctx.enter_context(tc.tile_pool(name="at", bufs=4))
        p_tr = ctx.enter_context(tc.tile_pool(name="tr", bufs=2))
        p_y = ctx.enter_context(tc.tile_pool(name="ysb", bufs=2))
        p_xr = ctx.enter_context(tc.tile_pool(name="xr", bufs=2))
        p_st = ctx.enter_context(tc.tile_pool(name="st", bufs=2))
        ps1 = ctx.enter_context(tc.tile_pool(name="ps1", bufs=4, space="PSUM"))
        ps2 = ctx.enter_context(tc.tile_pool(name="ps2", bufs=2, space="PSUM"))
        pskv = ctx.enter_context(tc.tile_pool(name="pskv", bufs=2, space="PSUM"))

        # ---- weight/const DMAs on the ACT HWDGE queue ----
        wv = p_w.tile([128, KD, HID], E4, tag="w")
        nc.scalar.dma_start(wv, ap["wv8"].rearrange("(kt p) e -> p kt e", p=128))
        wqk_sb = consts.tile([128, KD, DQK], BF, tag="wqk")
        nc.scalar.dma_start(wqk_sb, ap["wqk"].rearrange("(kt p) q -> p kt q", p=128))
        xhalo = consts.tile([128, KD, 128], E4, tag="xhalo")
        nc.scalar.dma_start(xhalo, ap["xhalo8"].rearrange("(kt p) t -> p kt t", p=128))
        bqk = consts.tile([128, 1], F32, tag="bqk")
        nc.scalar.dma_start(bqk, ap["bqk"])

        # fp8 x for stream 0 (v GEMM)
        x8 = p_x8.tile([128, KD, T], E4, tag="x8")
        for q in range(2):
            nc.sync.dma_start(
                x8[:, q * 4:(q + 1) * 4, :],
                ap["xh8"][q * 512:(q + 1) * 512, :].rearrange(
                    "(kt p) t -> p kt t", p=128))

        def load_xt(s):
            halves = []
            for h in range(2):
                t = p_xt.tile([128, KD // 2, T], BF, tag="xt")
                for q in range(2):
                    r0 = h * 512 + q * 256
                    nc.sync.dma_start(
                        t[:, q * 2:(q + 1) * 2, :],
                        ap["xt"][s, r0:r0 + 256, :].rearrange(
                            "(kt p) t -> p kt t", p=128))
                halves.append(t)
            return halves

        def xt_k(halves, kt):
            return halves[kt // 4][:, kt % 4, :]

        x_s0 = load_xt(0)

        # warm-up collective: absorbs first-collective setup latency
        warm = consts.tile([128, 16], BF, tag="warm")
        nc.vector.memset(warm, 0.0)
        nc.sync.dma_start(ap["cc_warm_in"], warm)
        nc.gpsimd.collective_compute(
            "AllGather", ALU.bypass, replica_groups=[list(range(NC))],
            ins=[ap["cc_warm_in"]], outs=[ap["cc_warm_out"]])

        # ---- v = silu(x8 @ wv)/WS  (1-term fp8 DR), stored e4m3 ----
        if WITH_VBIAS or WITH_OBIAS:
            ones_t = consts.tile([1, 128], BF, tag="ones")
            nc.vector.memset(ones_t, 1.0)
        if WITH_VBIAS:
            wvb = consts.tile([1, HID], BF, tag="wvb")
            nc.scalar.dma_start(wvb, ap["wvb"])

        x_s3 = load_xt(3)
        v8 = p_v.tile([128, TT, HID], E4, tag="v")
        for tt in range(TT):
            for c in range(4):
                c0 = c * 512
                ps = ps1.tile([128, 512], F32, tag="ps1")
                for kp in range(KP):
                    nc.tensor.matmul(
                        ps, x8[:, 2 * kp:2 * kp + 2, tt * 128:(tt + 1) * 128],
                        wv[:, 2 * kp:2 * kp + 2, c0:c0 + 512],
                        start=(kp == 0), stop=(kp == KP - 1 and not WITH_VBIAS),
                        perf_mode=DR, skip_group_check=True)
                if WITH_VBIAS:
                    nc.tensor.matmul(ps, ones_t[0:1, 0:128],
                                     wvb[0:1, c0:c0 + 512],
                                     start=False, stop=True,
                                     skip_group_check=True)
                # ACT writes bf16; DVE does the e4m3 cast (ACT fp8 output
                # untested on this hw)
                stg = p_st.tile([128, 512], BF, tag="st")
                nc.scalar.activation(stg, ps, AF.Silu, bias=0.0, scale=1.0 / WS)
                nc.vector.tensor_copy(v8[:, tt, c0:c0 + 512], stg)

        # ---- qk streams (bf16); s0 and s3 first (kv needs lk from s3) ----
        qkT = p_qkT.tile([128, 4, T], BF, tag="qkT")

        def qk_stream(s, halves):
            for ch in range(2):
                ps = ps1.tile([128, 512], F32, tag="ps1")
                for kt in range(KD):
                    nc.tensor.matmul(
                        ps, wqk_sb[:, kt, :],
                        xt_k(halves, kt)[:, ch * 512:(ch + 1) * 512],
                        start=(kt == 0), stop=(kt == KD - 1))
                nc.scalar.activation(qkT[:, s, ch * 512:(ch + 1) * 512], ps,
                                     AF.Silu, bias=bqk, scale=1.0)

        qk_stream(0, x_s0)
        qk_stream(3, x_s3)

        # lk (stream 3) token-major via PE transpose, stored e4m3
        ident = consts.tile([128, 128], BF, tag="ident")
        make_identity(nc, ident)
        lk8 = p_lk.tile([128, TT, 128], E4, tag="lk")
        for tt in range(TT):
            pt = ps2.tile([128, 128], BF, tag="ps2")
            nc.tensor.transpose(pt, qkT[:, 3, tt * 128:(tt + 1) * 128], ident)
            nc.vector.tensor_copy(lk8[:, tt, :], pt)

        # ---- kv sweep (fp8 DR over token-tile pairs) + snapshots ----
        # local full-sum totals stage in S_full[:, 0, :] (dead until the
        # post-AllGather weighted sum overwrites it)
        S_full = p_S.tile([128, NG, HID], BF, tag="S")

        for half in range(2):
            pk0 = pskv.tile([128, 512], F32, tag="pskv")
            pk1 = pskv.tile([128, 512], F32, tag="pskv")
            pk = [pk0, pk1]
            for g in range(NG):
                for c in range(2):
                    c0 = half * 1024 + c * 512
                    nc.tensor.matmul(pk[c], lk8[:, 2 * g:2 * g + 2, :],
                                     v8[:, 2 * g:2 * g + 2, c0:c0 + 512],
                                     start=(g == 0), stop=(g == NG - 1),
                                     perf_mode=DR, skip_group_check=True)
                    dst = S_full[:, (g + 1) % NG, c0:c0 + 512]
                    nc.scalar.activation(dst, pk[c], AF.Copy, bias=0.0,
                                         scale=1.0 / G)

        # AllGather the per-core totals
        nc.sync.dma_start(ap["cc_in"], S_full[:, 0, :])
        nc.gpsimd.collective_compute(
            "AllGather", ALU.bypass, replica_groups=[list(range(NC))],
            ins=[ap["cc_in"]], outs=[ap["cc_out"]])

        # ---- halo tail (last 32 tokens of previous core, padded to 128
        # tokens so the DR shapes match the v chains), masked on core 0 ----
        tails = p_tails.tile([32, NG, HID], E4, tag="tails")
        hmask = consts.tile([32, 1], F32, tag="hmask")
        nc.scalar.dma_start(hmask, ap["hmask"])
        for c in range(4):
            c0 = c * 512
            ps = ps1.tile([128, 512], F32, tag="ps1")
            for kp in range(KP):
                nc.tensor.matmul(ps, xhalo[:, 2 * kp:2 * kp + 2, :],
                                 wv[:, 2 * kp:2 * kp + 2, c0:c0 + 512],
                                 start=(kp == 0),
                                 stop=(kp == KP - 1 and not WITH_VBIAS),
                                 perf_mode=DR, skip_group_check=True)
            if WITH_VBIAS:
                nc.tensor.matmul(ps, ones_t[0:1, 0:128], wvb[0:1, c0:c0 + 512],
                                 start=False, stop=True, skip_group_check=True)
            stg = p_st.tile([128, 512], BF, tag="st")
            nc.scalar.activation(stg, ps, AF.Silu, bias=0.0, scale=1.0 / WS)
            nc.vector.tensor_scalar_mul(tails[:, 0, c0:c0 + 512],
                                        stg[0:32, :], hmask)
        # group tails: last 32 tokens of the previous group (partition remap)
        for g in range(1, NG):
            nc.sync.dma_start(tails[:, g, :], v8[96:128, 2 * g - 1, :])

        # ---- remaining qk streams ----
        x_s2 = load_xt(2)
        qk_stream(2, x_s2)
        x_s1 = load_xt(1)
        qk_stream(1, x_s1)

        # ---- sim / attn8 (conv band folded in, scaled by ASC, e4m3) ----
        triu = consts.tile([128, 128], BF, tag="triu")
        nc.scalar.dma_start(triu, ap["triu"])
        band0 = consts.tile([128, 256], BF, tag="band0")
        nc.scalar.dma_start(band0, ap["band0"])
        band1 = consts.tile([128, 128], BF, tag="band1")
        nc.scalar.dma_start(band1, ap["band1"])
        bprev = consts.tile([32, 32], E4, tag="bprev")
        nc.scalar.dma_start(bprev, ap["bprev"])

        attn8s = []
        for g in range(NG):
            i0 = g * G
            a0 = p_a0.tile([128, 256], BF, tag="a0")
            ps = ps2.tile([128, 256], F32, tag="ps2")
            nc.tensor.matmul(ps, qkT[:, 2, i0:i0 + 128], qkT[:, 0, i0:i0 + 256],
                             start=True, stop=True)
            nc.scalar.activation(a0, ps, AF.Relu, bias=0.0, scale=RASC / G)
            nc.vector.tensor_mul(a0[:, 0:128], a0[:, 0:128], triu)

            a1 = p_a1.tile([128, 128], BF, tag="a1")
            ps = ps2.tile([128, 256], F32, tag="ps2")
            nc.tensor.matmul(ps[:, 0:128], qkT[:, 2, i0 + 128:i0 + 256],
                             qkT[:, 0, i0 + 128:i0 + 256], start=True, stop=True)
            nc.scalar.activation(a1, ps[:, 0:128], AF.Relu, bias=0.0,
                                 scale=RASC / G)
            nc.vector.tensor_mul(a1, a1, triu)

            t8 = p_at.tile([128, 2, 256], E4, tag="at")
            sq = p_sq.tile([128, 256], BF, tag="sq")
            nc.vector.tensor_mul(sq, a0, a0)
            nc.vector.scalar_tensor_tensor(t8[:, 0, :], sq, 1.0, band0,
                                           op0=ALU.mult, op1=ALU.add)
            nc.vector.tensor_scalar_mul(t8[:, 1, 0:128], triu, 0.0)
            sq1 = p_sq.tile([128, 256], BF, tag="sq")
            nc.vector.tensor_mul(sq1[:, 0:128], a1, a1)
            nc.vector.scalar_tensor_tensor(t8[:, 1, 128:256], sq1[:, 0:128],
                                           1.0, band1, op0=ALU.mult, op1=ALU.add)
            attn8s.append(t8)

        # ---- quad + conv -> outT (fp8 DR + 32-col tail matmul) ----
        outT = p_outT.tile([128, ET, T], BF, tag="outT")
        for g in range(NG):
            for et in range(ET):
                e0 = et * 128
                po = ps2.tile([128, 256], F32, tag="ps2")
                nc.tensor.matmul(po, v8[:, 2 * g:2 * g + 2, e0:e0 + 128],
                                 attn8s[g], start=True, stop=False,
                                 perf_mode=DR, skip_group_check=True)
                nc.tensor.matmul(po[:, 0:32], tails[:, g, e0:e0 + 128],
                                 bprev[0:32, :], start=False, stop=True,
                                 skip_group_check=True)
                nc.scalar.activation(outT[:, et, g * G:(g + 1) * G], po,
                                     AF.Copy, bias=0.0, scale=1.0 / ASC)

        # ---- weighted prefix sum of AllGathered totals (DVE), emitted
        # early so it runs concurrent with quad/gate PE work ----
        wsumw = consts.tile([128, NC], F32, tag="wsumw")
        nc.scalar.dma_start(wsumw, ap["wsumw"])
        for r in range(NC):
            for h in range(2):
                h0 = h * 1024
                trt = p_tr.tile([128, 1024], BF, tag="tr")
                nc.sync.dma_start(
                    trt, ap["cc_out"][r * 128:(r + 1) * 128, h0:h0 + 1024])
                if r == 0:
                    nc.vector.tensor_scalar_mul(S_full[:, 0, h0:h0 + 1024],
                                                trt, wsumw[:, 0:1])
                else:
                    nc.vector.scalar_tensor_tensor(
                        S_full[:, 0, h0:h0 + 1024], trt, wsumw[:, r:r + 1],
                        S_full[:, 0, h0:h0 + 1024], op0=ALU.mult, op1=ALU.add)
        for g in range(1, NG):
            nc.vector.tensor_add(S_full[:, g, :], S_full[:, g, :],
                                 S_full[:, 0, :])

        # ---- gate weights + consts (DMA while PE chews on quad) ----
        wg_sb = p_w.tile([128, KD, HID], BF, tag="w")
        for kt in range(KD):
            nc.scalar.dma_start(wg_sb[:, kt, :],
                                ap["wg"][kt * 128:(kt + 1) * 128, :])
        bgate = consts.tile([128, ET], F32, tag="bgate")
        nc.scalar.dma_start(bgate, ap["bgate"])

        def gate_th(s, halves, th, og, defer_muls=False):
            t0 = th * 512
            for et in range(ET):
                ps = ps1.tile([128, 512], F32, tag="ps1")
                for kt in range(KD):
                    nc.tensor.matmul(
                        ps, wg_sb[:, kt, et * 128:(et + 1) * 128],
                        xt_k(halves, kt)[:, t0:t0 + 512],
                        start=(kt == 0), stop=(kt == KD - 1))
                nc.scalar.activation(og[:, et, :], ps, AF.Silu,
                                     bias=bgate[:, et:et + 1], scale=1.0)
                if not defer_muls:
                    nc.vector.tensor_mul(og[:, et, :], og[:, et, :],
                                         outT[:, et, t0:t0 + 512])

        xg0 = load_xt(0)
        og0 = p_v.tile([128, ET, 512], BF, tag="v")
        gate_th(0, xg0, 0, og0, defer_muls=True)

        # scheduler fence: keep the lin matmuls (which wait on the
        # AllGather) from being hoisted ahead of the pre-fence PE work
        tc.no_sync_barrier()

        # ---- lin: outT += S_full[g] @ lq, et-major with og0 muls chasing ----
        for et in range(ET):
            e0 = et * 128
            for g in range(NG):
                po = ps2.tile([128, 256], F32, tag="ps2")
                nc.tensor.matmul(po, S_full[:, g, e0:e0 + 128],
                                 qkT[:, 1, g * G:(g + 1) * G],
                                 start=True, stop=True)
                nc.vector.tensor_add(outT[:, et, g * G:(g + 1) * G],
                                     outT[:, et, g * G:(g + 1) * G], po)
            nc.vector.tensor_mul(og0[:, et, :], og0[:, et, :],
                                 outT[:, et, 0:512])

        if DEBUG_DUMPS:
            nc.sync.dma_start(ap["dbg_v8"], v8)
            nc.sync.dma_start(ap["dbg_qkT"], qkT)
            nc.sync.dma_start(ap["dbg_outT"], outT)
            nc.sync.dma_start(ap["dbg_sfull"], S_full)
            nc.sync.dma_start(ap["dbg_tails"], tails)
            for g in range(NG):
                nc.sync.dma_start(ap["dbg_attn8"][g], attn8s[g])

        # ---- y projection (bf16) ----
        wout_sb = p_S.tile([128, ET, DIM], BF, tag="S")
        for kt in range(ET):
            nc.scalar.dma_start(wout_sb[:, kt, :],
                                ap["wout"][kt * 128:(kt + 1) * 128, :])
        if WITH_OBIAS:
            bout = consts.tile([1, DIM], BF, tag="bout")
            nc.scalar.dma_start(bout, ap["bout"])

        def y_th(s, og, th):
            for tl in range(4):
                tt = th * 4 + tl
                for nch in range(2):
                    n0 = nch * 512
                    ps = ps1.tile([128, 512], F32, tag="ps1")
                    for kt in range(ET):
                        nc.tensor.matmul(
                            ps, og[:, kt, tl * 128:(tl + 1) * 128],
                            wout_sb[:, kt, n0:n0 + 512],
                            start=(kt == 0),
                            stop=(kt == ET - 1 and not WITH_OBIAS),
                            skip_group_check=True)
                    if WITH_OBIAS:
                        nc.tensor.matmul(ps, ones_t[0:1, 0:128],
                                         bout[0:1, n0:n0 + 512],
                                         start=False, stop=True,
                                         skip_group_check=True)
                    xr = p_xr.tile([128, 512], F32, tag="xr")
                    nc.sync.dma_start(
                        xr, ap["xtok"][s, tt * 128:(tt + 1) * 128, n0:n0 + 512])
                    ysb = p_y.tile([128, 512], F32, tag="ysb")
                    nc.vector.scalar_tensor_tensor(
                        ysb, ps, 0.0, xr, op0=ALU.add, op1=ALU.add)
                    nc.sync.dma_start(
                        ap["y"][s, tt * 128:(tt + 1) * 128, n0:n0 + 512], ysb)

        # gate s0-th1 runs on PE while DVE finishes the og0 muls
        og1 = p_v.tile([128, ET, 512], BF, tag="v")
        gate_th(0, xg0, 1, og1)
        y_th(0, og0, 0)
        y_th(0, og1, 1)
        xg = {}
        for s in range(1, 4):
            halves = xg[s] if s in xg else load_xt(s)
            if s + 1 < 4:
                xg[s + 1] = load_xt(s + 1)
            for th in range(2):
                og = p_v.tile([128, ET, 512], BF, tag="v")
                gate_th(s, halves, th, og)
                y_th(s, og, th)


def build_nc(with_vbias=None, with_obias=None):
    global WITH_VBIAS, WITH_OBIAS
    if with_vbias is not None:
        WITH_VBIAS = with_vbias
    if with_obias is not None:
        WITH_OBIAS = with_obias
    nc = bacc.Bacc("TRN2", target_bir_lowering=False, debug=False, num_devices=NC)
    ap = {}

    def dram(name, shape, dt, kind=None, addr_space=None):
        kw = {}
        if kind:
            kw["kind"] = kind
        if addr_space:
            kw["addr_space"] = addr_space
        ap[name] = nc.dram_tensor(name, shape, dt, **kw).ap()

    dram("xt", [4, DIM, T], BF, kind="ExternalInput")
    dram("xh8", [DIM, T], E4, kind="ExternalInput")
    dram("xhalo8", [DIM, 128], E4, kind="ExternalInput")
    dram("xtok", [4, T, DIM], F32, kind="ExternalInput")
    dram("wv8", [DIM, HID], E4, kind="ExternalInput")
    dram("wg", [DIM, HID], BF, kind="ExternalInput")
    dram("wqk", [DIM, DQK], BF, kind="ExternalInput")
    dram("wout", [HID, DIM], BF, kind="ExternalInput")
    dram("wvb", [1, HID], BF, kind="ExternalInput")
    dram("bout", [1, DIM], BF, kind="ExternalInput")
    dram("bgate", [128, ET], F32, kind="ExternalInput")
    dram("bqk", [128, 1], F32, kind="ExternalInput")
    dram("triu", [128, 128], BF, kind="ExternalInput")
    dram("band0", [128, 256], BF, kind="ExternalInput")
    dram("band1", [128, 128], BF, kind="ExternalInput")
    dram("bprev", [32, 32], E4, kind="ExternalInput")
    dram("hmask", [32, 1], F32, kind="ExternalInput")
    dram("wsumw", [128, NC], F32, kind="ExternalInput")
    if DEBUG_DUMPS:
        dram("dbg_v8", [128, TT, HID], E4, kind="ExternalOutput")
        dram("dbg_qkT", [128, 4, T], BF, kind="ExternalOutput")
        dram("dbg_outT", [128, ET, T], BF, kind="ExternalOutput")
        dram("dbg_sfull", [128, NG, HID], BF, kind="ExternalOutput")
        dram("dbg_tails", [32, NG, HID], E4, kind="ExternalOutput")
        dram("dbg_attn8", [NG, 128, 2, 256], E4, kind="ExternalOutput")
    dram("cc_warm_in", [128, 16], BF)
    dram("cc_warm_out", [NC * 128, 16], BF, addr_space="Shared")
    dram("cc_in", [128, HID], BF)
    dram("cc_out", [NC * 128, HID], BF, addr_space="Shared")
    dram("y", [4, T, DIM], F32, kind="ExternalOutput")

    with tile.TileContext(nc) as tc:
        _emit(tc, ap)
    nc.compile()
    return nc


def host_prep(inputs):
    """Pure layout transforms: shard, transpose, fp8 casts, conv-band consts."""
    x = np.ascontiguousarray(np.asarray(inputs["x"], np.float32)[0])  # [4,N,DIM]
    W_h = np.asarray(inputs["W_h"], np.float32)
    b_h = np.asarray(inputs["b_h"], np.float32)
    W_qk = np.asarray(inputs["W_qk"], np.float32)
    b_qk = np.asarray(inputs["b_qk"], np.float32)
    W_out = np.asarray(inputs["W_out"], np.float32)
    b_out = np.asarray(inputs["b_out"], np.float32)
    cw = np.asarray(inputs["conv_w"], np.float32)

    jj = np.arange(128)[:, None]
    ii = np.arange(128)[None, :]
    d = ii - jj
    triu = (ii >= jj).astype(bf16)
    w63 = cw * (np.arange(63) <= 31)
    bdiag = np.where((d >= 0) & (d <= 31), w63[np.clip(31 - d, 0, 62)], 0.0)
    dc = (ii + 128) - jj
    bcorn = np.where((dc >= 0) & (dc <= 31), w63[np.clip(31 - dc, 0, 62)], 0.0)
    band0 = (np.concatenate([bdiag, bcorn], axis=1) * ASC).astype(bf16)
    band1 = (bdiag * ASC).astype(bf16)
    jt = np.arange(32)[:, None]
    ip = np.arange(32)[None, :]
    dp = ip + 32 - jt
    bprev = np.where((dp >= 1) & (dp <= 31),
                     w63[np.clip(31 - dp, 0, 62)], 0.0) * ASC

    common = {
        "wv8": (np.ascontiguousarray(W_h[:, :HID]) * WS).astype(e4m3),
        "wg": np.ascontiguousarray(W_h[:, HID:]).astype(bf16),
        "wqk": W_qk.astype(bf16),
        "wout": W_out.astype(bf16),
        "wvb": (b_h[None, :HID] * WS).astype(bf16),
        "bout": b_out[None, :].astype(bf16),
        "bgate": np.ascontiguousarray(
            b_h[HID:].reshape(ET, 128).T).astype(np.float32),
        "bqk": b_qk[:, None].astype(np.float32),
        "triu": triu, "band0": band0, "band1": band1,
        "bprev": bprev.astype(e4m3),
    }

    in_maps = []
    for c in range(NC):
        sl = slice(c * T, (c + 1) * T)
        x_c = x[:, sl, :]
        xt = np.ascontiguousarray(x_c.transpose(0, 2, 1))   # [4, DIM, T]
        xh = np.zeros((DIM, 128), np.float32)
        if c > 0:
            xh[:, 0:32] = x[0, c * T - 32:c * T, :].T
        m = dict(common)
        m["xt"] = xt.astype(bf16)
        m["xh8"] = xt[0].astype(e4m3)
        m["xhalo8"] = xh.astype(e4m3)
        m["xtok"] = np.ascontiguousarray(x_c)
        m["hmask"] = np.full((32, 1), 1.0 if c > 0 else 0.0, np.float32)
        w = np.zeros((128, NC), np.float32)
        w[:, :c] = 1.0
        m["wsumw"] = w
        in_maps.append(m)
    return in_maps


_NC_PROG = None
_NC_FLAGS = None


def kernel(**inputs):
    global _NC_PROG, _NC_FLAGS
    b_h = np.asarray(inputs["b_h"], np.float32)
    b_out = np.asarray(inputs["b_out"], np.float32)
    flags = (bool(np.any(b_h[:HID])), bool(np.any(b_out)))
    if _NC_PROG is None or _NC_FLAGS != flags:
        _NC_PROG = build_nc(with_vbias=flags[0], with_obias=flags[1])
        _NC_FLAGS = flags
    in_maps = host_prep(inputs)
    res = run_bass_kernel_spmd(_NC_PROG, in_maps, list(range(NC)))
    y = np.stack([res.results[c]["y"] for c in range(NC)], axis=1)
    return np.ascontiguousarray(y.reshape(4, NSEQ, DIM)[None]).astype(np.float32)
